# revision 15
# baseline (speedup 1.0000x reference)
"""Trainium2 Bass kernel for nn_AttnDecoderRNN (B=512,S=512,H=128,OUT=136,T=8).

Data-parallel over batch: 8 cores x 64 batch rows. Each core runs the full
T=8 recurrent attention-GRU loop on its shard; no collectives.

Per-core layout strategy:
  - keys^T  [h,s] per b (bf16)  -> rhs for keys_proj precompute (PE)
  - keysC   [s%128, (chunk,h)] per b (bf16) -> stationary for context matmuls
  - keys_proj kp [h,s] per b (bf16, 8MB) precomputed once on PE
  - per step, per half (32 b):
      q = 0.5*Wa^T.T @ h2           (PE, psum)
      tanh(kp_b + q_b)              (ACT, bias-fused, 32 instrs = the wall)
      scores^T = tanhed_chunk.T @ Va (PE M=1, dense psum cols)
      scores   = PE-transpose blocks -> [32b, 512s] psum
      softmax  = exp(accum_out=sum) + reciprocal + scale (ACT/DVE)
      w^T      = PE-transpose -> bf16
      ctx^T    = keysC_chunk.T @ w^T chunk (PE M=1, accumulated)
      GRU      = 11 small PE matmuls + ACT tanh-as-sigmoid + DVE elementwise
  - halves alternate so one half's glue overlaps the other half's tanh.
"""

import os
import sys

import numpy as np

for _p in ("/opt/trn_rl_repo", "/root/.axon_site/_ro/trn_rl_repo"):
    if os.path.isdir(_p) and _p not in sys.path:
        sys.path.insert(0, _p)

import ml_dtypes  # noqa: E402

import concourse.bass as bass  # noqa: E402
import concourse.mybir as mybir  # noqa: E402
import concourse.tile as tile  # noqa: E402
from concourse import bacc  # noqa: E402
from concourse.bass_utils import run_bass_kernel_spmd  # noqa: E402
from concourse.masks import make_identity  # noqa: E402

F32 = mybir.dt.float32
BF16 = mybir.dt.bfloat16
AF = mybir.ActivationFunctionType
OP = mybir.AluOpType

H = 128
OUT = 136
T = int(os.environ.get("KT", "8"))
B = 512
S = 512
NCORES = 8
BSH = B // NCORES          # 64 batch rows per core
HALF = BSH // 2            # 32
NCH = S // 128             # 4 s-chunks

_cache = {}


def _build():
    nc = bacc.Bacc(None, target_bir_lowering=False)

    # ---------------- DRAM I/O ----------------
    keysT_d = nc.dram_tensor("keysT", [BSH, H, S], BF16, kind="ExternalInput")
    keysC_d = nc.dram_tensor("keysC", [BSH, 128, NCH * H], BF16, kind="ExternalInput")
    ehT_d = nc.dram_tensor("ehT", [2, H, BSH], F32, kind="ExternalInput")
    tpoT_d = nc.dram_tensor("tpoT", [OUT, BSH], F32, kind="ExternalInput")

    uaT_d = nc.dram_tensor("uaT", [H, H], BF16, kind="ExternalInput")
    va_d = nc.dram_tensor("va", [H, 1], BF16, kind="ExternalInput")
    waTh_d = nc.dram_tensor("waTh", [H, H], F32, kind="ExternalInput")
    wihT_d = nc.dram_tensor("wihT", [2 * H, 3 * H], F32, kind="ExternalInput")
    whhTh_d = nc.dram_tensor("whhTh", [H, 3 * H], F32, kind="ExternalInput")
    outWTh_d = nc.dram_tensor("outWTh", [H, OUT], F32, kind="ExternalInput")
    embWT_d = nc.dram_tensor("embWT", [OUT, H], F32, kind="ExternalInput")
    # bias columns
    qb_d = nc.dram_tensor("qb", [H, 1], F32, kind="ExternalInput")      # Wa_b+Ua_b
    embb_d = nc.dram_tensor("embb", [H, 1], F32, kind="ExternalInput")
    brzh_d = nc.dram_tensor("brzh", [2 * H, 1], F32, kind="ExternalInput")  # 0.5*(b_ih+b_hh)[r,z]
    bihn_d = nc.dram_tensor("bihn", [H, 1], F32, kind="ExternalInput")
    bhhn_d = nc.dram_tensor("bhhn", [H, 1], F32, kind="ExternalInput")
    outb_d = nc.dram_tensor("outb", [OUT, 1], F32, kind="ExternalInput")

    dec_d = nc.dram_tensor("dec", [T, OUT, BSH], F32, kind="ExternalOutput")
    attn_d = nc.dram_tensor("attn", [T, BSH, S], F32, kind="ExternalOutput")
    hid_d = nc.dram_tensor("hid", [H, BSH], F32, kind="ExternalOutput")

    with tile.TileContext(nc) as tc:
        with (
            tc.tile_pool(name="const", bufs=1) as cp,
            tc.tile_pool(name="kp", bufs=BSH) as kpP,
            tc.tile_pool(name="ks", bufs=BSH) as ksP,
            tc.tile_pool(name="kt", bufs=8) as ktP,
            tc.tile_pool(name="th", bufs=34) as thP,
            tc.tile_pool(name="sm", bufs=3) as smP,
            tc.tile_pool(name="psA", bufs=2, space=bass.MemorySpace.PSUM) as psA,
            tc.tile_pool(name="psB", bufs=2, space=bass.MemorySpace.PSUM) as psB,
            tc.tile_pool(name="psC", bufs=2, space=bass.MemorySpace.PSUM) as psC,
            tc.tile_pool(name="psK", bufs=2, space=bass.MemorySpace.PSUM) as psK,
        ):
            # ---------- constants ----------
            ident = cp.tile([128, 128], F32)
            make_identity(nc, ident[:])

            uaT = cp.tile([H, H], BF16)
            va = cp.tile([H, 1], BF16)
            waTh = cp.tile([H, H], F32)
            wihT0 = cp.tile([128, 3 * H], F32)
            wihT1 = cp.tile([128, 3 * H], F32)
            whhTh = cp.tile([H, 3 * H], F32)
            outWTh = cp.tile([H, OUT], F32)
            embWT0 = cp.tile([128, H], F32)
            embWT1 = cp.tile([8, H], F32)
            qb = cp.tile([H, 1], F32)
            embb = cp.tile([H, 1], F32)
            brz_r = cp.tile([H, 1], F32)
            brz_z = cp.tile([H, 1], F32)
            bihn = cp.tile([H, 1], F32)
            bhhn = cp.tile([H, 1], F32)
            outb0 = cp.tile([128, 1], F32)
            outb1 = cp.tile([8, 1], F32)
            ehT0 = cp.tile([H, BSH], F32)
            ehT1 = cp.tile([H, BSH], F32)
            tpoT0 = cp.tile([128, BSH], F32)
            tpoT1 = cp.tile([8, BSH], F32)

            dma = nc.sync.dma_start
            dma(uaT[:], uaT_d[:])
            dma(va[:], va_d[:])
            dma(waTh[:], waTh_d[:])
            dma(wihT0[:], wihT_d[0:128, :])
            dma(wihT1[:], wihT_d[128:256, :])
            dma(whhTh[:], whhTh_d[:])
            dma(outWTh[:], outWTh_d[:])
            dma(embWT0[:], embWT_d[0:128, :])
            dma(embWT1[:], embWT_d[128:136, :])
            dma(qb[:], qb_d[:])
            dma(embb[:], embb_d[:])
            dma(brz_r[:], brzh_d[0:H, :])
            dma(brz_z[:], brzh_d[H:2 * H, :])
            dma(bihn[:], bihn_d[:])
            dma(bhhn[:], bhhn_d[:])
            dma(outb0[:], outb_d[0:128, :])
            dma(outb1[:], outb_d[128:136, :])
            dma(ehT0[:], ehT_d[0])
            dma(ehT1[:], ehT_d[1])
            dma(tpoT0[:], tpoT_d[0:128, :])
            dma(tpoT1[:], tpoT_d[128:136, :])

            # ---------- preamble compute ----------
            # h2_0 = eh0+eh1 (= 2*h0), per half
            h2 = [smP.tile([H, HALF], F32, tag="h2a", name="h2a"),
                  smP.tile([H, HALF], F32, tag="h2b", name="h2b")]
            nc.vector.tensor_tensor(h2[0][:], ehT0[:, 0:HALF], ehT1[:, 0:HALF], OP.add)
            nc.vector.tensor_tensor(h2[1][:], ehT0[:, HALF:BSH], ehT1[:, HALF:BSH], OP.add)

            # embT = emb_W @ tpo + emb_b   [128, 64]
            pemb = psK.tile([H, BSH], F32, tag="kp")
            nc.tensor.matmul(pemb[:], embWT0[:], tpoT0[:], start=True, stop=False,
                             skip_group_check=True)
            nc.tensor.matmul(pemb[:], embWT1[:], tpoT1[:], start=False, stop=True,
                             skip_group_check=True)
            embT = cp.tile([H, BSH], F32)
            nc.vector.tensor_scalar_add(embT[:], pemb[:], embb[:])

            # keys stream in; kp = Ua @ keysT  (per b), evac to bf16
            kp_t = []
            ks_t = []
            for b in range(BSH):
                kt = ktP.tile([H, S], BF16, tag="kt")
                dma(kt[:], keysT_d[b])
                ks = ksP.tile([128, NCH * H], BF16, tag="ks")
                dma(ks[:], keysC_d[b])
                ks_t.append(ks)
                pk = psK.tile([H, S], F32, tag="kp")
                nc.tensor.matmul(pk[:], uaT[:], kt[:], start=True, stop=True,
                                 skip_group_check=True)
                kp = kpP.tile([H, S], BF16, tag="kp")
                if b % 2 == 0:
                    nc.vector.tensor_copy(kp[:], pk[:])
                else:
                    nc.scalar.activation(kp[:], pk[:], AF.Copy)
                kp_t.append(kp)

            # ---------- recurrent loop ----------
            for t in range(T):
                for h in range(2):
                    b0 = h * HALF
                    pm = psC.tile([128, 512], F32, tag="misc")
                    r_q = pm[:, 0:32]
                    r_wT = pm[:, 32:160]
                    r_ctx = pm[:, 160:192]
                    r_r = pm[:, 192:224]
                    r_z = pm[:, 224:256]
                    r_gin = pm[:, 256:288]
                    r_ghn = pm[:, 288:320]
                    r_o1 = pm[:, 320:352]
                    r_o2 = pm[0:8, 352:384]

                    # q = 0.5*Wa^T.T @ h2 (+WaUa bias at evac)
                    nc.tensor.matmul(r_q, waTh[:], h2[h][:], start=True, stop=True,
                                     skip_group_check=True)
                    qT = smP.tile([H, HALF], F32, tag="qT")
                    nc.vector.tensor_scalar_add(qT[:], r_q, qb[:])

                    # tanh(kp_b + q_b) then scores^T columns
                    pscT = psA.tile([128, 128], F32, tag="scT")
                    for bh in range(HALF):
                        b = b0 + bh
                        th_t = thP.tile([H, S], BF16, tag="th")
                        nc.scalar.activation(th_t[:], kp_t[b][:], AF.Tanh,
                                             bias=qT[:, bh:bh + 1])
                        for c in range(NCH):
                            nc.tensor.matmul(
                                pscT[:, c * 32 + bh:c * 32 + bh + 1],
                                th_t[:, c * 128:(c + 1) * 128],
                                va[:],
                                start=(bh == 0 and c == 0),
                                stop=(bh == HALF - 1 and c == NCH - 1),
                                skip_group_check=True,
                            )
                    scT = smP.tile([128, 128], F32, tag="scTs")
                    nc.vector.tensor_copy(scT[:], pscT[:])

                    # transpose to [32b, 512s]
                    psc = psB.tile([32, 512], F32, tag="sc")
                    for c in range(NCH):
                        nc.tensor.matmul(
                            psc[:, c * 128:(c + 1) * 128],
                            scT[:, c * 32:(c + 1) * 32],
                            ident[:],
                            is_transpose=True,
                            start=True, stop=True,
                            skip_group_check=True,
                        )

                    # softmax (no max-sub needed: |scores| <= sum|Va| ~ 5)
                    exps = smP.tile([32, 512], F32, tag="exps")
                    sums = smP.tile([32, 1], F32, tag="sums")
                    nc.scalar.activation(exps[:], psc[:], AF.Exp, accum_out=sums[:])
                    rec = smP.tile([32, 1], F32, tag="rec")
                    nc.vector.reciprocal(rec[:], sums[:])
                    w = smP.tile([32, 512], F32, tag="w")
                    nc.vector.tensor_scalar_mul(w[:], exps[:], rec[:])
                    dma(attn_d[t, b0:b0 + HALF, :], w[:])

                    # w^T (bf16) for context
                    for c in range(NCH):
                        nc.tensor.matmul(
                            r_wT[:, c * 32:(c + 1) * 32],
                            w[:, c * 128:(c + 1) * 128],
                            ident[0:32, 0:32],
                            is_transpose=True,
                            start=True, stop=True,
                            skip_group_check=True,
                        )
                    wT = smP.tile([128, 128], BF16, tag="wT")
                    nc.vector.tensor_copy(wT[:], r_wT)

                    # context^T columns, contracted over s chunks
                    for bh in range(HALF):
                        b = b0 + bh
                        for c in range(NCH):
                            nc.tensor.matmul(
                                r_ctx[:, bh:bh + 1],
                                ks_t[b][:, c * 128:(c + 1) * 128],
                                wT[:, c * 32 + bh:c * 32 + bh + 1],
                                start=(c == 0),
                                stop=(c == NCH - 1),
                                skip_group_check=True,
                            )
                    ctxT = smP.tile([H, HALF], F32, tag="ctxT")
                    nc.vector.tensor_copy(ctxT[:], r_ctx)

                    # GRU gates: gi = W_ih @ [emb;ctx], gh = 0.5*W_hh @ h2
                    emb_h = embT[:, b0:b0 + HALF]
                    for gidx, reg in ((0, r_r), (1, r_z), (2, r_gin)):
                        g0 = gidx * H
                        nc.tensor.matmul(reg, wihT0[:, g0:g0 + H], emb_h,
                                         start=True, stop=False, skip_group_check=True)
                        nc.tensor.matmul(reg, wihT1[:, g0:g0 + H], ctxT[:],
                                         start=False, stop=(gidx == 2),
                                         skip_group_check=True)
                        if gidx < 2:
                            nc.tensor.matmul(reg, whhTh[:, g0:g0 + H], h2[h][:],
                                             start=False, stop=True,
                                             skip_group_check=True)
                    nc.tensor.matmul(r_ghn, whhTh[:, 2 * H:3 * H], h2[h][:],
                                     start=True, stop=True, skip_group_check=True)

                    # r,z = sigmoid -> tanh(0.5x) trick; h kept doubled (h2=2h)
                    tr = smP.tile([H, HALF], F32, tag="tr")
                    nc.scalar.activation(tr[:], r_r, AF.Tanh, bias=brz_r[:], scale=0.5)
                    tz = smP.tile([H, HALF], F32, tag="tz")
                    nc.scalar.activation(tz[:], r_z, AF.Tanh, bias=brz_z[:], scale=0.5)

                    ghnb = smP.tile([H, HALF], F32, tag="ghnb")
                    nc.vector.tensor_scalar_add(ghnb[:], r_ghn, bhhn[:])
                    t1 = smP.tile([H, HALF], F32, tag="t1")
                    nc.vector.tensor_tensor(t1[:], tr[:], ghnb[:], OP.mult)
                    t2 = smP.tile([H, HALF], F32, tag="t2")
                    nc.vector.tensor_tensor(t2[:], t1[:], ghnb[:], OP.add)  # 2*r*ghnb
                    t3 = smP.tile([H, HALF], F32, tag="t3")
                    nc.vector.scalar_tensor_tensor(t3[:], r_gin, 2.0, t2[:],
                                                   OP.mult, OP.add)  # 2(gi_n + r*ghnb)
                    n_t = smP.tile([H, HALF], F32, tag="n")
                    nc.scalar.activation(n_t[:], t3[:], AF.Tanh, bias=bihn[:], scale=0.5)

                    s2 = smP.tile([H, HALF], F32, tag="s2")
                    nc.vector.scalar_tensor_tensor(s2[:], h2[h][:], 0.5, n_t[:],
                                                   OP.mult, OP.subtract)  # 0.5h2 - n
                    s4 = smP.tile([H, HALF], F32, tag="s4")
                    nc.vector.scalar_tensor_tensor(s4[:], h2[h][:], 0.5, n_t[:],
                                                   OP.mult, OP.add)       # 0.5h2 + n
                    s3 = smP.tile([H, HALF], F32, tag="s3")
                    nc.vector.tensor_tensor(s3[:], tz[:], s2[:], OP.mult)
                    h2n = smP.tile([H, HALF], F32, tag="h2a" if h == 0 else "h2b",
                                   name="h2n")
                    nc.vector.tensor_tensor(h2n[:], s4[:], s3[:], OP.add)
                    h2[h] = h2n

                    # out = 0.5*out_W @ h2_new + out_b
                    nc.tensor.matmul(r_o1, outWTh[:, 0:128], h2n[:],
                                     start=True, stop=True, skip_group_check=True)
                    nc.tensor.matmul(r_o2, outWTh[:, 128:136], h2n[:],
                                     start=True, stop=True, skip_group_check=True)
                    o1 = smP.tile([128, HALF], F32, tag="o1")
                    nc.vector.tensor_scalar_add(o1[:], r_o1, outb0[:])
                    o2 = smP.tile([8, HALF], F32, tag="o2")
                    nc.vector.tensor_scalar_add(o2[:], r_o2, outb1[:])
                    dma(dec_d[t, 0:128, b0:b0 + HALF], o1[:])
                    dma(dec_d[t, 128:136, b0:b0 + HALF], o2[:])

            # final hidden: h = 0.5*h2
            hf = cp.tile([H, BSH], F32)
            nc.vector.tensor_scalar_mul(hf[:, 0:HALF], h2[0][:], 0.5)
            nc.vector.tensor_scalar_mul(hf[:, HALF:BSH], h2[1][:], 0.5)
            dma(hid_d[:], hf[:])

    nc.finalize()
    return nc


def _prep_inputs(inputs):
    enc = np.ascontiguousarray(inputs["encoder_outputs"], dtype=np.float32)
    eh = np.ascontiguousarray(inputs["encoder_hidden"], dtype=np.float32)
    tpo = np.ascontiguousarray(inputs["target_preceeding_output"], dtype=np.float32)
    emb_W = np.asarray(inputs["emb_W"], np.float32)
    emb_b = np.asarray(inputs["emb_b"], np.float32)
    Wa = np.asarray(inputs["Wa"], np.float32)
    Wa_b = np.asarray(inputs["Wa_b"], np.float32)
    Ua = np.asarray(inputs["Ua"], np.float32)
    Ua_b = np.asarray(inputs["Ua_b"], np.float32)
    Va = np.asarray(inputs["Va"], np.float32)
    W_ih = np.asarray(inputs["W_ih"], np.float32)
    W_hh = np.asarray(inputs["W_hh"], np.float32)
    b_ih = np.asarray(inputs["b_ih"], np.float32)
    b_hh = np.asarray(inputs["b_hh"], np.float32)
    out_W = np.asarray(inputs["out_W"], np.float32)
    out_b = np.asarray(inputs["out_b"], np.float32)

    bf = ml_dtypes.bfloat16
    shared = {
        "uaT": np.ascontiguousarray(Ua.T).astype(bf),
        "va": np.ascontiguousarray(Va.reshape(H, 1)).astype(bf),
        "waTh": np.ascontiguousarray(0.5 * Wa.T),
        "wihT": np.ascontiguousarray(W_ih.T),
        "whhTh": np.ascontiguousarray(0.5 * W_hh.T),
        "outWTh": np.ascontiguousarray(0.5 * out_W.T),
        "embWT": np.ascontiguousarray(emb_W.T),
        "qb": np.ascontiguousarray((Wa_b + Ua_b).reshape(H, 1)),
        "embb": np.ascontiguousarray(emb_b.reshape(H, 1)),
        "brzh": np.ascontiguousarray((0.5 * (b_ih + b_hh))[0:2 * H].reshape(2 * H, 1)),
        "bihn": np.ascontiguousarray(b_ih[2 * H:3 * H].reshape(H, 1)),
        "bhhn": np.ascontiguousarray(b_hh[2 * H:3 * H].reshape(H, 1)),
        "outb": np.ascontiguousarray(out_b.reshape(OUT, 1)),
    }

    in_maps = []
    for c in range(NCORES):
        sl = slice(c * BSH, (c + 1) * BSH)
        k = enc[sl]                                    # [64, 512, 128]
        keysT = np.ascontiguousarray(k.transpose(0, 2, 1)).astype(bf)
        keysC = np.ascontiguousarray(
            k.reshape(BSH, NCH, 128, H).transpose(0, 2, 1, 3).reshape(BSH, 128, NCH * H)
        ).astype(bf)
        m = dict(shared)
        m["keysT"] = keysT
        m["keysC"] = keysC
        m["ehT"] = np.ascontiguousarray(eh[:, sl, :].transpose(0, 2, 1))
        m["tpoT"] = np.ascontiguousarray(tpo[sl, 0, :].T)
        in_maps.append(m)
    return in_maps


def _get_nc():
    if "nc" not in _cache:
        _cache["nc"] = _build()
    return _cache["nc"]


def _postprocess(results):
    dec = np.concatenate([r["dec"].transpose(2, 0, 1) for r in results], axis=0)
    attn = np.concatenate([r["attn"].transpose(1, 0, 2) for r in results], axis=0)
    hid = np.concatenate([r["hid"].T for r in results], axis=0)[None]
    return dec, hid, attn


def run(inputs, trace=False):
    nc = _get_nc()
    in_maps = _prep_inputs(inputs)
    res = run_bass_kernel_spmd(nc, in_maps, list(range(NCORES)), trace=trace)
    return _postprocess(res.results), res


def sim_exec_time_ns():
    """Cost-model timeline of the exact BIR (CoreSim with production cost model)."""
    import ml_dtypes as md
    from concourse.bass_interp import CoreSim

    nc = _get_nc()
    sim = CoreSim(nc, trace=False)
    rng = np.random.default_rng(0)
    specs = [
        ("keysT", (BSH, H, S), md.bfloat16), ("keysC", (BSH, 128, NCH * H), md.bfloat16),
        ("ehT", (2, H, BSH), np.float32), ("tpoT", (OUT, BSH), np.float32),
        ("uaT", (H, H), md.bfloat16), ("va", (H, 1), md.bfloat16),
        ("waTh", (H, H), np.float32), ("wihT", (2 * H, 3 * H), np.float32),
        ("whhTh", (H, 3 * H), np.float32), ("outWTh", (H, OUT), np.float32),
        ("embWT", (OUT, H), np.float32), ("qb", (H, 1), np.float32),
        ("embb", (H, 1), np.float32), ("brzh", (2 * H, 1), np.float32),
        ("bihn", (H, 1), np.float32), ("bhhn", (H, 1), np.float32),
        ("outb", (OUT, 1), np.float32),
    ]
    for name, shape, dt in specs:
        sim.tensor(name)[:] = (rng.standard_normal(shape) * 0.05).astype(dt)
    sim.simulate()
    return int(sim.time)


def kernel(**inputs):
    (dec, hid, attn), _ = run(inputs, trace=False)
    return dec, hid, attn
